# revision 62
# baseline (speedup 1.0000x reference)
"""Trainium2 Bass kernel for nn_Attention_17334488007364.

Computation (per batch element, x as [C=128, N=4096]):
    q = wq @ x                      [16, 4096]
    k = maxpool2(wk @ x)            [16, 1024]
    v = maxpool2(wv @ x)            [64, 1024]
    attn = softmax(q^T k, axis=m)   [4096, 1024]
    o = v @ attn^T                  [64, 4096]
    out = gamma * (wo @ o) + x      [128, 4096]

Sharding: pure data parallel -- B=16 over 8 cores, 2 batch elements/core.
Measured: ~163-175us exec per core (neuron-profile), rel err 1.6e-4.

Per-core dataflow (chunk-major, pooled positions m on partitions):
  - fused qkv projection (bf16) with a 32-aligned packed weight layout;
    2x2 maxpool on DVE straight out of the PSUM drain
  - 16 global n-chunk iterations (2 batches x 8 chunks of 512). Scores
    for a chunk pack 4 DIFFERENT m-tiles into the PE's 4 row groups via
    tile_position (each k strip holds the full pooled k), 2 matmuls per
    2-bank score buffer, exp on ACT straight out of PSUM ([128,1024]
    calls; ACT is the bottleneck: 8.4M exps/core ~= 67us busy)
  - AV for chunk c accumulates sum_m [v^T | 1].T @ p_c; row 64 of the
    accumulator is the softmax denominator for free. AV+epilogue of
    older chunks are emission-interleaved between score chunks via
    explicit queues (per-engine streams execute in emission order, so
    this IS the schedule); epilogues wait 2 iterations so the wo matmul
    never head-of-line-blocks PE on the rden round-trip
  - denominators of 4 chunks are DMA-packed to partitions 0..3 and
    inverted by ONE native DVE reciprocal (its cost is ~6.4 cyc/elem
    regardless of partition count); 1/den broadcast via one 3D-AP DRAM
    round-trip DMA; onorm = (ou*gamma)*rden in one scalar_tensor_tensor,
    then the wo matmul and a fused residual add
  - DMA descriptor ISSUE costs ~600ns on the owning sequencer, so DMA
    count is minimized and spread: x loads on ACT's sequencer, outputs +
    gamma on gpsimd, replication/epilogue on sync
"""

from contextlib import ExitStack

import numpy as np

import concourse.bacc as bacc
import concourse.mybir as mybir
from concourse import masks
from concourse.alu_op_type import AluOpType
from concourse.tile import TileContext

FP32 = mybir.dt.float32
BF16 = mybir.dt.bfloat16
AFT = mybir.ActivationFunctionType

# Per-core problem shape (hardcoded; harness provides full inputs).
B_FULL, C, H, W = 16, 128, 64, 64
N_CORES = 8
B_LOC = B_FULL // N_CORES            # 2
HW = H * W                           # 4096
M = HW // 4                          # 1024 (after 2x2 maxpool)
CQ, CV = C // 8, C // 2              # 16, 64
NCH = 512                            # psum-bank-sized n chunk
NCHUNKS = HW // NCH                  # 8
EXPSPAN = 2048                       # 4 banks per exp call
MT = M // 128                        # 8 m-tiles of 128

PACK = True                          # 4-way row-group packing for scores


def build_nc():
    nc = bacc.Bacc()
    x_e = nc.declare_dram_parameter("x", [B_LOC, C, HW], FP32, isOutput=False)
    wq_e = nc.declare_dram_parameter("wq", [CQ, C], FP32, isOutput=False)
    wk_e = nc.declare_dram_parameter("wk", [CQ, C], FP32, isOutput=False)
    wv_e = nc.declare_dram_parameter("wv", [CV, C], FP32, isOutput=False)
    wo_e = nc.declare_dram_parameter("wo", [C, CV], FP32, isOutput=False)
    g_e = nc.declare_dram_parameter("gamma", [1], FP32, isOutput=False)
    out_e = nc.declare_dram_parameter("out", [B_LOC, C, HW], FP32, isOutput=True)

    with TileContext(nc) as tc, ExitStack() as ctx:
        const = ctx.enter_context(tc.tile_pool(name="const", bufs=1))
        xpool = ctx.enter_context(tc.tile_pool(name="x", bufs=2))
        qkv = ctx.enter_context(tc.tile_pool(name="qkv", bufs=2))
        ppool = ctx.enter_context(tc.tile_pool(name="p", bufs=9))
        vtpool = ctx.enter_context(tc.tile_pool(name="vt", bufs=18))
        eppool = ctx.enter_context(tc.tile_pool(name="ep", bufs=3))
        outpool = ctx.enter_context(tc.tile_pool(name="outp", bufs=3))
        # PSUM budget (8 banks): scores 4 + av 2 + w 1 + m 1
        ps_s = ctx.enter_context(tc.tile_pool(name="ps_s", bufs=3, space="PSUM"))
        ps_av = ctx.enter_context(tc.tile_pool(name="ps_av", bufs=1, space="PSUM"))
        ps_w = ctx.enter_context(tc.tile_pool(name="ps_w", bufs=1, space="PSUM"))
        dscratch = ctx.enter_context(tc.tile_pool(name="dscr", bufs=4, space="DRAM"))

        # ---------------- constants / weight preprocessing ----------------
        ident = const.tile([128, 128], FP32)
        masks.make_identity(nc, ident[:])
        ident_bf = const.tile([128, 128], BF16)
        masks.make_identity(nc, ident_bf[:])

        wq_sb = const.tile([CQ, C], FP32, tag="wq")
        wk_sb = const.tile([CQ, C], FP32, tag="wk")
        wv_sb = const.tile([CV, C], FP32, tag="wv")
        wo_sb = const.tile([C, CV], FP32, tag="wo")
        nc.sync.dma_start(wq_sb[:], wq_e[:])
        nc.sync.dma_start(wk_sb[:], wk_e[:])
        nc.sync.dma_start(wv_sb[:], wv_e[:])
        nc.sync.dma_start(wo_sb[:], wo_e[:])

        # gamma broadcast to all 128 partitions: [128, 1]
        g_sb = const.tile([128, 1], FP32, tag="g")
        nc.gpsimd.dma_start(
            g_sb[:, 0:1], g_e[:].unsqueeze(0).partition_broadcast(128)
        )

        # W_cat^T: cols 0:16 = wq^T, 32:48 = wk^T, 64:128 = wv^T (32-aligned
        # so PSUM consumer slices start at partition 0/32/64)
        ps_wt = ps_w.tile([128, NCH], FP32, tag="wm")
        nc.tensor.transpose(ps_wt[:, 0:CQ], wq_sb[:], ident[0:CQ, 0:CQ])
        nc.tensor.transpose(ps_wt[:, 32 : 32 + CQ], wk_sb[:], ident[0:CQ, 0:CQ])
        nc.tensor.transpose(ps_wt[:, 64 : 64 + CV], wv_sb[:], ident[0:CV, 0:CV])
        wcatT = const.tile([128, 128], BF16, tag="wcatT")
        nc.vector.memset(wcatT[:], 0.0)
        nc.vector.tensor_copy(wcatT[:, 0:CQ], ps_wt[:, 0:CQ])
        nc.vector.tensor_copy(wcatT[:, 32 : 32 + CQ], ps_wt[:, 32 : 32 + CQ])
        nc.vector.tensor_copy(wcatT[:, 64 : 64 + CV], ps_wt[:, 64 : 64 + CV])

        # wo^T [64, 128] bf16 (lhsT for output projection)
        ps_wo = ps_w.tile([128, NCH], FP32, tag="wm")
        nc.tensor.transpose(ps_wo[0:CV, 0:C], wo_sb[:], ident[:])
        woT = const.tile([CV, C], BF16, tag="woT")
        nc.vector.tensor_copy(woT[:], ps_wo[0:CV, 0:C])

        # ---------------- chunk-major pipeline ----------------
        # 16 global chunk iterations (2 batches x 8 n-chunks). Scores for a
        # chunk pack 4 DIFFERENT m-tiles into the 4 PE row groups (each k
        # strip holds the full pooled k), so the AV accumulation for chunk c
        # only lags its own exps -- no half/batch phase barriers.

        def load_x(b):
            # f32 loads on the (idle at startup) ACT sequencer
            x_sb = xpool.tile([C, HW], FP32, tag="x", name=f"x_{b}")
            for cc in range(NCHUNKS):
                csl = slice(cc * NCH, (cc + 1) * NCH)
                if b == 0 and cc == 0:
                    for s in range(2):
                        nc.scalar.dma_start(
                            x_sb[64 * s : 64 * (s + 1), csl],
                            x_e[b, 64 * s : 64 * (s + 1), csl],
                        )
                else:
                    nc.scalar.dma_start(x_sb[:, csl], x_e[b, :, csl])
            return x_sb

        def prep_init(b, x_sb):
            st = {
                "b": b,
                "x_sb": x_sb,
                "x_bf": qkv.tile([C, HW], BF16, tag="xbf", bufs=2, name=f"xbf_{b}"),
                "qkv_full": qkv.tile([C, HW], BF16, tag="qkvfull", name=f"qf_{b}"),
                "q_rep": qkv.tile([128, HW], BF16, tag="qrep", name=f"qr_{b}"),
                "kv_sb": qkv.tile([128, M], BF16, tag="k", name=f"kv_{b}"),
                "k_rep": qkv.tile([128, M], BF16, tag="krep", name=f"kr_{b}"),
                "vT": [None] * MT,
            }
            return st

        def prep_chunk(st, cc, with_vt=True):
            b = st["b"]
            x_sb, x_bf = st["x_sb"], st["x_bf"]
            qkv_full, kv_sb = st["qkv_full"], st["kv_sb"]
            sl = slice(cc * NCH, (cc + 1) * NCH)
            nc.vector.tensor_copy(x_bf[:, sl], x_sb[:, sl])
            ps_p = ps_w.tile([128, NCH], FP32, tag="wm", name=f"pj_{b}_{cc}")
            nc.tensor.matmul(ps_p[:], wcatT[:], x_bf[:, sl], start=True, stop=True)
            # single PSUM->SBUF drain; pooling runs from SBUF (only one PSUM
            # read operand is legal per DVE op)
            nc.vector.tensor_copy(qkv_full[:, sl], ps_p[:])
            # maxpool 2x2: h-pairs first (contiguous last dim), then w-pairs;
            # k (rows 32:48) and v (rows 64:128) pool separately (engine APs
            # from base!=0 cannot span blocks)
            kv1 = qkv.tile([128, 4 * 64], BF16, tag="kv1", name=f"kv1_{b}_{cc}")
            for lo, hi in ((32, 32 + CQ), (64, 128)):
                pp = qkv_full[lo:hi, sl].rearrange(
                    "p (h2 two w) -> p h2 two w", h2=4, two=2, w=64
                )
                s1 = kv1[lo:hi, :].rearrange("p (h w) -> p h w", h=4, w=64)
                nc.vector.tensor_tensor(
                    s1, pp[:, :, 0, :], pp[:, :, 1, :], AluOpType.max
                )
                s1w = kv1[lo:hi, :].rearrange(
                    "p (h w2 two) -> p h w2 two", h=4, w2=32, two=2
                )
                s2 = kv_sb[lo:hi, cc * 128 : (cc + 1) * 128].rearrange(
                    "p (h w2) -> p h w2", h=4, w2=32
                )
                nc.vector.tensor_tensor(
                    s2, s1w[:, :, :, 0], s1w[:, :, :, 1], AluOpType.max
                )
            if with_vt:
                emit_vt(st, cc)
            # replication (strip 1 of k is native kv_sb rows 32:48):
            # chunk-granular for the first chunks so scores start right
            # after projection chunk 0; half-granular afterwards
            if cc < 3:
                ksl2 = slice(cc * 128, (cc + 1) * 128)
                for s in (0, 2, 3):
                    nc.sync.dma_start(
                        st["k_rep"][32 * s : 32 * s + CQ, ksl2],
                        kv_sb[32 : 32 + CQ, ksl2],
                    )
                for s in range(1, 4):
                    nc.sync.dma_start(
                        st["q_rep"][32 * s : 32 * s + CQ, sl], qkv_full[0:CQ, sl]
                    )
            elif cc in (3, 7):
                h = cc // 4
                kh = slice(3 * 128 if h == 0 else h * M // 2, (h + 1) * M // 2)
                hsl = slice(3 * NCH if h == 0 else h * EXPSPAN, (h + 1) * EXPSPAN)
                for s in (0, 2, 3):
                    nc.sync.dma_start(
                        st["k_rep"][32 * s : 32 * s + CQ, kh], kv_sb[32 : 32 + CQ, kh]
                    )
                for s in range(1, 4):
                    nc.sync.dma_start(
                        st["q_rep"][32 * s : 32 * s + CQ, hsl], qkv_full[0:CQ, hsl]
                    )

        def emit_vt(st, j):
            # vT~ strip j: [128, 65] bf16, col 64 = ones (v at kv_sb rows
            # 64:128; identity block rows 64:128 matches the base partition)
            b, kv_sb = st["b"], st["kv_sb"]
            ps_t = ps_av.tile([128, NCH * 2], BF16, tag="av", name=f"tp_{b}_{j}")
            nc.tensor.transpose(
                ps_t[:, 0:CV],
                kv_sb[64:128, j * 128 : (j + 1) * 128],
                ident_bf[64:128, 64:128],
            )
            vt = vtpool.tile([128, CV + 1], BF16, tag="vt", name=f"vt_{b}_{j}")
            nc.vector.tensor_copy(vt[:, 0:CV], ps_t[:, 0:CV])
            nc.vector.memset(vt[:, CV : CV + 1], 1.0)
            st["vT"][j] = vt

        def scores_chunk(st, c, pc):
            # 8 m-tiles of chunk c as 4 issues of 2 packed matmuls; adjacent
            # issues use disjoint row-group pairs so 4 run concurrently
            b = st["b"]
            qkv_full, q_rep = st["qkv_full"], st["q_rep"]
            kv_sb, k_rep = st["kv_sb"], st["k_rep"]
            ncol = c * NCH
            for j in range(4):
                s_ps = ps_s.tile([128, 2 * NCH], FP32, tag="s", name=f"s_{b}_{c}_{j}")
                for i in range(2):
                    t = 2 * j + i
                    rg = 2 * (j % 2) + i
                    ksrc = kv_sb if rg == 1 else k_rep
                    qsrc = qkv_full if rg == 0 else q_rep
                    nc.tensor.matmul(
                        s_ps[:, i * NCH : (i + 1) * NCH],
                        ksrc[32 * rg : 32 * rg + CQ, t * 128 : (t + 1) * 128],
                        qsrc[32 * rg : 32 * rg + CQ, ncol : ncol + NCH],
                        start=True,
                        stop=True,
                        tile_position=(32 * rg, 0),
                    )
                nc.scalar.activation(
                    pc[:, (2 * j) * NCH : (2 * j + 2) * NCH], s_ps[:], AFT.Exp
                )

        def av_chunk(st, ep, c, pc):
            b, vT = st["b"], st["vT"]
            o_ps = ps_av.tile([128, NCH], FP32, tag="av", name=f"av_{b}_{c}")
            for t in range(MT):
                nc.tensor.matmul(
                    o_ps[0 : CV + 1, :],
                    vT[t][:],
                    pc[:, t * NCH : (t + 1) * NCH],
                    start=(t == 0),
                    stop=(t == MT - 1),
                )
            # single drain: rows 0:64 = unnormalized AV, row 64 = denominator
            ou = eppool.tile([CV + 1, NCH], BF16, tag="ou", bufs=8,
                             name=f"ou_{b}_{c}")
            nc.vector.tensor_copy(ou[:], o_ps[0 : CV + 1, :])
            nc.sync.dma_start(ep["dstage"][c % 4 : c % 4 + 1, :], ou[CV : CV + 1, :])
            ep["ou"][c % 4] = ou

        def den_chain(ep):
            b, h = ep["bh"]
            rdn = eppool.tile([4, NCH], FP32, tag="rdn", bufs=2, name=f"rdn_{b}_{h}")
            nc.vector.reciprocal(rdn[:], ep["dstage"][:])
            ep["rd4"] = dscratch.tile([4, NCH], FP32, tag="rd", name=f"rd4_{b}_{h}")
            nc.sync.dma_start(ep["rd4"][:], rdn[:])

        def epilogue_chunk(st, ep, c):
            b, h = ep["bh"]
            x_sb = st["x_sb"]
            sl = slice(c * NCH, (c + 1) * NCH)
            if "den" not in ep:
                ep["den"] = eppool.tile([CV, 4 * NCH], FP32, tag="den", bufs=2,
                                        name=f"den_{b}_{h}")
                nc.sync.dma_start(
                    ep["den"][:].rearrange("p (c n) -> p c n", c=4, n=NCH),
                    ep["rd4"][:].partition_broadcast(CV),
                )
            onorm = eppool.tile([CV, NCH], BF16, tag="onorm", bufs=3,
                                name=f"on_{b}_{c}")
            nc.vector.scalar_tensor_tensor(
                onorm[:],
                ep["ou"][c % 4][0:CV, :],
                g_sb[0:CV, 0:1],
                ep["den"][:, (c % 4) * NCH : (c % 4 + 1) * NCH],
                AluOpType.mult,
                AluOpType.mult,
            )
            # residual folded into the wo matmul: identity-matmul x into the
            # PSUM bank (start=True), then accumulate wo@onorm on top, so the
            # DVE drain is a plain copy instead of a tensor add
            o2_ps = ps_w.tile([128, NCH], FP32, tag="wm", name=f"o2_{b}_{c}")
            nc.tensor.matmul(
                o2_ps[:], ident_bf[:], st["x_bf"][:, sl], start=True, stop=False
            )
            nc.tensor.matmul(o2_ps[:], woT[:], onorm[:], start=False, stop=True)
            out_sb = outpool.tile([C, NCH], FP32, tag="out", name=f"os_{b}_{c}")
            nc.vector.tensor_copy(out_sb[:], o2_ps[:])
            nc.gpsimd.dma_start(out_e[b, :, sl], out_sb[:])

        # ---- emission ----
        x0 = load_x(0)
        x1 = load_x(1)
        st0 = prep_init(0, x0)
        st1 = prep_init(1, x1)
        for cc in range(NCHUNKS):
            prep_chunk(st0, cc, with_vt=True)
        sts = {0: st0, 1: st1}

        av_q = []      # (st, ep, c, pc) awaiting AV
        ep_q = []      # (st, ep, c) awaiting epilogue
        eps = {}
        for i in range(2 * NCHUNKS):
            b, c = i // 8, i % 8
            st = sts[b]
            g = (b, c // 4)
            if g not in eps:
                eps[g] = {
                    "bh": g,
                    "dstage": eppool.tile([4, NCH], BF16, tag="dstage", bufs=3,
                                          name=f"dst_{g[0]}_{g[1]}"),
                    "ou": {},
                }
            if 4 <= i <= 7:
                prep_chunk(st1, 2 * (i - 4), with_vt=True)
                prep_chunk(st1, 2 * (i - 4) + 1, with_vt=True)
            pc = ppool.tile([128, HW], BF16, tag="pc", name=f"pc_{b}_{c}")
            scores_chunk(st, c, pc)
            av_q.append((st, eps[g], c, pc))
            if i >= 6:
                pops = min(2 if len(av_q) >= 2 else 1, len(av_q))
                for _ in range(pops):
                    pst, pep, pcn, ppc = av_q.pop(0)
                    av_chunk(pst, pep, pcn, ppc)
                    if pcn % 4 == 3:
                        den_chain(pep)
                        for ec in range(pcn - 3, pcn + 1):
                            ep_q.append((pst, pep, ec, i + 1))
            if ep_q and ep_q[0][3] <= i:
                e = ep_q.pop(0)
                epilogue_chunk(e[0], e[1], e[2])
        # drain remaining work
        while av_q:
            pst, pep, pcn, ppc = av_q.pop(0)
            av_chunk(pst, pep, pcn, ppc)
            if pcn % 4 == 3:
                den_chain(pep)
                for ec in range(pcn - 3, pcn + 1):
                    ep_q.append((pst, pep, ec, 0))
        while ep_q:
            e = ep_q.pop(0)
            epilogue_chunk(e[0], e[1], e[2])

    nc.finalize()
    return nc


_NC_CACHE = None


def _get_nc():
    global _NC_CACHE
    if _NC_CACHE is None:
        _NC_CACHE = build_nc()
    return _NC_CACHE


def kernel(**inputs) -> np.ndarray:
    from concourse.bass_utils import run_bass_kernel_spmd

    x = np.asarray(inputs["x"], dtype=np.float32).reshape(B_FULL, C, HW)
    wq = np.asarray(inputs["wq"], dtype=np.float32)
    wk = np.asarray(inputs["wk"], dtype=np.float32)
    wv = np.asarray(inputs["wv"], dtype=np.float32)
    wo = np.asarray(inputs["wo"], dtype=np.float32)
    gamma = np.asarray(inputs["gamma"], dtype=np.float32)

    nc = _get_nc()
    in_maps = []
    for i in range(N_CORES):
        in_maps.append(
            {
                "x": np.ascontiguousarray(x[i * B_LOC : (i + 1) * B_LOC]),
                "wq": wq,
                "wk": wk,
                "wv": wv,
                "wo": wo,
                "gamma": gamma,
            }
        )
    res = run_bass_kernel_spmd(nc, in_maps, core_ids=list(range(N_CORES)))
    outs = [res.results[i]["out"].reshape(B_LOC, C, H, W) for i in range(N_CORES)]
    return np.concatenate(outs, axis=0)


if __name__ == "__main__":
    import reference

    inputs = {k: np.asarray(v) for k, v in reference.setup_inputs().items()}
    expected = np.asarray(reference.reference(**inputs))
    actual = kernel(**inputs)
    err = np.linalg.norm(actual - expected) / np.linalg.norm(expected)
    print("Relative error:", err)


# revision 63
# speedup vs baseline: 1.0284x; 1.0284x over previous
"""Trainium2 Bass kernel for nn_Attention_17334488007364.

Computation (per batch element, x as [C=128, N=4096]):
    q = wq @ x                      [16, 4096]
    k = maxpool2(wk @ x)            [16, 1024]
    v = maxpool2(wv @ x)            [64, 1024]
    attn = softmax(q^T k, axis=m)   [4096, 1024]
    o = v @ attn^T                  [64, 4096]
    out = gamma * (wo @ o) + x      [128, 4096]

Sharding: pure data parallel -- B=16 over 8 cores, 2 batch elements/core.
Measured: ~163-175us exec per core (neuron-profile), rel err 1.6e-4.

Per-core dataflow (chunk-major, pooled positions m on partitions):
  - fused qkv projection (bf16) with a 32-aligned packed weight layout;
    2x2 maxpool on DVE straight out of the PSUM drain
  - 16 global n-chunk iterations (2 batches x 8 chunks of 512). Scores
    for a chunk pack 4 DIFFERENT m-tiles into the PE's 4 row groups via
    tile_position (each k strip holds the full pooled k), 2 matmuls per
    2-bank score buffer, exp on ACT straight out of PSUM ([128,1024]
    calls; ACT is the bottleneck: 8.4M exps/core ~= 67us busy)
  - AV for chunk c accumulates sum_m [v^T | 1].T @ p_c; row 64 of the
    accumulator is the softmax denominator for free. AV+epilogue of
    older chunks are emission-interleaved between score chunks via
    explicit queues (per-engine streams execute in emission order, so
    this IS the schedule); epilogues wait 2 iterations so the wo matmul
    never head-of-line-blocks PE on the rden round-trip
  - denominators of 4 chunks are DMA-packed to partitions 0..3 and
    inverted by ONE native DVE reciprocal (its cost is ~6.4 cyc/elem
    regardless of partition count); 1/den broadcast via one 3D-AP DRAM
    round-trip DMA; onorm = (ou*gamma)*rden in one scalar_tensor_tensor,
    then the wo matmul and a fused residual add
  - DMA descriptor ISSUE costs ~600ns on the owning sequencer, so DMA
    count is minimized and spread: x loads on ACT's sequencer, outputs +
    gamma on gpsimd, replication/epilogue on sync
"""

from contextlib import ExitStack

import numpy as np

import concourse.bacc as bacc
import concourse.mybir as mybir
from concourse import masks
from concourse.alu_op_type import AluOpType
from concourse.tile import TileContext

FP32 = mybir.dt.float32
BF16 = mybir.dt.bfloat16
AFT = mybir.ActivationFunctionType

# Per-core problem shape (hardcoded; harness provides full inputs).
B_FULL, C, H, W = 16, 128, 64, 64
N_CORES = 8
B_LOC = B_FULL // N_CORES            # 2
HW = H * W                           # 4096
M = HW // 4                          # 1024 (after 2x2 maxpool)
CQ, CV = C // 8, C // 2              # 16, 64
NCH = 512                            # psum-bank-sized n chunk
NCHUNKS = HW // NCH                  # 8
EXPSPAN = 2048                       # 4 banks per exp call
MT = M // 128                        # 8 m-tiles of 128

PACK = True                          # 4-way row-group packing for scores


def build_nc():
    nc = bacc.Bacc()
    x_e = nc.declare_dram_parameter("x", [B_LOC, C, HW], FP32, isOutput=False)
    wq_e = nc.declare_dram_parameter("wq", [CQ, C], FP32, isOutput=False)
    wk_e = nc.declare_dram_parameter("wk", [CQ, C], FP32, isOutput=False)
    wv_e = nc.declare_dram_parameter("wv", [CV, C], FP32, isOutput=False)
    wo_e = nc.declare_dram_parameter("wo", [C, CV], FP32, isOutput=False)
    g_e = nc.declare_dram_parameter("gamma", [1], FP32, isOutput=False)
    out_e = nc.declare_dram_parameter("out", [B_LOC, C, HW], FP32, isOutput=True)

    with TileContext(nc) as tc, ExitStack() as ctx:
        const = ctx.enter_context(tc.tile_pool(name="const", bufs=1))
        xpool = ctx.enter_context(tc.tile_pool(name="x", bufs=2))
        qkv = ctx.enter_context(tc.tile_pool(name="qkv", bufs=2))
        ppool = ctx.enter_context(tc.tile_pool(name="p", bufs=9))
        vtpool = ctx.enter_context(tc.tile_pool(name="vt", bufs=18))
        eppool = ctx.enter_context(tc.tile_pool(name="ep", bufs=3))
        outpool = ctx.enter_context(tc.tile_pool(name="outp", bufs=3))
        # PSUM budget (8 banks): scores 4 + av 2 + w 1 + m 1
        ps_s = ctx.enter_context(tc.tile_pool(name="ps_s", bufs=3, space="PSUM"))
        ps_av = ctx.enter_context(tc.tile_pool(name="ps_av", bufs=1, space="PSUM"))
        ps_w = ctx.enter_context(tc.tile_pool(name="ps_w", bufs=1, space="PSUM"))
        dscratch = ctx.enter_context(tc.tile_pool(name="dscr", bufs=4, space="DRAM"))

        # ---------------- constants / weight preprocessing ----------------
        ident = const.tile([128, 128], FP32)
        masks.make_identity(nc, ident[:])
        ident_bf = const.tile([128, 128], BF16)
        masks.make_identity(nc, ident_bf[:])

        wq_sb = const.tile([CQ, C], FP32, tag="wq")
        wk_sb = const.tile([CQ, C], FP32, tag="wk")
        wv_sb = const.tile([CV, C], FP32, tag="wv")
        wo_sb = const.tile([C, CV], FP32, tag="wo")
        nc.sync.dma_start(wq_sb[:], wq_e[:])
        nc.sync.dma_start(wk_sb[:], wk_e[:])
        nc.sync.dma_start(wv_sb[:], wv_e[:])
        nc.sync.dma_start(wo_sb[:], wo_e[:])

        # gamma broadcast to all 128 partitions: [128, 1]
        g_sb = const.tile([128, 1], FP32, tag="g")
        nc.gpsimd.dma_start(
            g_sb[:, 0:1], g_e[:].unsqueeze(0).partition_broadcast(128)
        )

        # W_cat^T: cols 0:16 = wq^T, 32:48 = wk^T, 64:128 = wv^T (32-aligned
        # so PSUM consumer slices start at partition 0/32/64)
        ps_wt = ps_w.tile([128, NCH], FP32, tag="wm")
        nc.tensor.transpose(ps_wt[:, 0:CQ], wq_sb[:], ident[0:CQ, 0:CQ])
        nc.tensor.transpose(ps_wt[:, 32 : 32 + CQ], wk_sb[:], ident[0:CQ, 0:CQ])
        nc.tensor.transpose(ps_wt[:, 64 : 64 + CV], wv_sb[:], ident[0:CV, 0:CV])
        wcatT = const.tile([128, 128], BF16, tag="wcatT")
        nc.vector.memset(wcatT[:], 0.0)
        nc.vector.tensor_copy(wcatT[:, 0:CQ], ps_wt[:, 0:CQ])
        nc.vector.tensor_copy(wcatT[:, 32 : 32 + CQ], ps_wt[:, 32 : 32 + CQ])
        nc.vector.tensor_copy(wcatT[:, 64 : 64 + CV], ps_wt[:, 64 : 64 + CV])

        # wo^T [64, 128] bf16 (lhsT for output projection)
        ps_wo = ps_w.tile([128, NCH], FP32, tag="wm")
        nc.tensor.transpose(ps_wo[0:CV, 0:C], wo_sb[:], ident[:])
        woT = const.tile([CV, C], BF16, tag="woT")
        nc.vector.tensor_copy(woT[:], ps_wo[0:CV, 0:C])

        # ---------------- chunk-major pipeline ----------------
        # 16 global chunk iterations (2 batches x 8 n-chunks). Scores for a
        # chunk pack 4 DIFFERENT m-tiles into the 4 PE row groups (each k
        # strip holds the full pooled k), so the AV accumulation for chunk c
        # only lags its own exps -- no half/batch phase barriers.

        def load_x(b):
            # f32 loads on the (idle at startup) ACT sequencer
            x_sb = xpool.tile([C, HW], FP32, tag="x", name=f"x_{b}")
            for cc in range(NCHUNKS):
                csl = slice(cc * NCH, (cc + 1) * NCH)
                if b == 0 and cc == 0:
                    for s in range(2):
                        nc.scalar.dma_start(
                            x_sb[64 * s : 64 * (s + 1), csl],
                            x_e[b, 64 * s : 64 * (s + 1), csl],
                        )
                else:
                    nc.scalar.dma_start(x_sb[:, csl], x_e[b, :, csl])
            return x_sb

        def prep_init(b, x_sb):
            st = {
                "b": b,
                "x_sb": x_sb,
                "x_bf": qkv.tile([C, HW], BF16, tag="xbf", bufs=2, name=f"xbf_{b}"),
                "qkv_full": qkv.tile([C, HW], BF16, tag="qkvfull", name=f"qf_{b}"),
                "q_rep": qkv.tile([128, HW], BF16, tag="qrep", name=f"qr_{b}"),
                "kv_sb": qkv.tile([128, M], BF16, tag="k", name=f"kv_{b}"),
                "k_rep": qkv.tile([128, M], BF16, tag="krep", name=f"kr_{b}"),
                "vT": [None] * MT,
            }
            return st

        def prep_chunk(st, cc, with_vt=True):
            b = st["b"]
            x_sb, x_bf = st["x_sb"], st["x_bf"]
            qkv_full, kv_sb = st["qkv_full"], st["kv_sb"]
            sl = slice(cc * NCH, (cc + 1) * NCH)
            nc.vector.tensor_copy(x_bf[:, sl], x_sb[:, sl])
            ps_p = ps_w.tile([128, NCH], FP32, tag="wm", name=f"pj_{b}_{cc}")
            nc.tensor.matmul(ps_p[:], wcatT[:], x_bf[:, sl], start=True, stop=True)
            # single PSUM->SBUF drain; pooling runs from SBUF (only one PSUM
            # read operand is legal per DVE op)
            nc.vector.tensor_copy(qkv_full[:, sl], ps_p[:])
            # maxpool 2x2: h-pairs first (contiguous last dim), then w-pairs;
            # k (rows 32:48) and v (rows 64:128) pool separately (engine APs
            # from base!=0 cannot span blocks)
            kv1 = qkv.tile([128, 4 * 64], BF16, tag="kv1", name=f"kv1_{b}_{cc}")
            for lo, hi in ((32, 32 + CQ), (64, 128)):
                pp = qkv_full[lo:hi, sl].rearrange(
                    "p (h2 two w) -> p h2 two w", h2=4, two=2, w=64
                )
                s1 = kv1[lo:hi, :].rearrange("p (h w) -> p h w", h=4, w=64)
                nc.vector.tensor_tensor(
                    s1, pp[:, :, 0, :], pp[:, :, 1, :], AluOpType.max
                )
                s1w = kv1[lo:hi, :].rearrange(
                    "p (h w2 two) -> p h w2 two", h=4, w2=32, two=2
                )
                s2 = kv_sb[lo:hi, cc * 128 : (cc + 1) * 128].rearrange(
                    "p (h w2) -> p h w2", h=4, w2=32
                )
                nc.vector.tensor_tensor(
                    s2, s1w[:, :, :, 0], s1w[:, :, :, 1], AluOpType.max
                )
            if with_vt:
                emit_vt(st, cc)
            # replication (strip 1 of k is native kv_sb rows 32:48):
            # chunk-granular for the first chunks so scores start right
            # after projection chunk 0; half-granular afterwards
            if cc < 3:
                ksl2 = slice(cc * 128, (cc + 1) * 128)
                for s in (0, 2, 3):
                    nc.sync.dma_start(
                        st["k_rep"][32 * s : 32 * s + CQ, ksl2],
                        kv_sb[32 : 32 + CQ, ksl2],
                    )
                for s in range(1, 4):
                    nc.sync.dma_start(
                        st["q_rep"][32 * s : 32 * s + CQ, sl], qkv_full[0:CQ, sl]
                    )
            elif cc in (3, 7):
                h = cc // 4
                kh = slice(3 * 128 if h == 0 else h * M // 2, (h + 1) * M // 2)
                hsl = slice(3 * NCH if h == 0 else h * EXPSPAN, (h + 1) * EXPSPAN)
                for s in (0, 2, 3):
                    nc.sync.dma_start(
                        st["k_rep"][32 * s : 32 * s + CQ, kh], kv_sb[32 : 32 + CQ, kh]
                    )
                for s in range(1, 4):
                    nc.sync.dma_start(
                        st["q_rep"][32 * s : 32 * s + CQ, hsl], qkv_full[0:CQ, hsl]
                    )

        def emit_vt(st, j):
            # vT~ strip j: [128, 65] bf16, col 64 = ones (v at kv_sb rows
            # 64:128; identity block rows 64:128 matches the base partition)
            b, kv_sb = st["b"], st["kv_sb"]
            ps_t = ps_av.tile([128, NCH * 2], BF16, tag="av", name=f"tp_{b}_{j}")
            nc.tensor.transpose(
                ps_t[:, 0:CV],
                kv_sb[64:128, j * 128 : (j + 1) * 128],
                ident_bf[64:128, 64:128],
            )
            vt = vtpool.tile([128, CV + 1], BF16, tag="vt", name=f"vt_{b}_{j}")
            nc.vector.tensor_copy(vt[:, 0:CV], ps_t[:, 0:CV])
            nc.vector.memset(vt[:, CV : CV + 1], 1.0)
            st["vT"][j] = vt

        def scores_chunk(st, c, pc):
            # 8 m-tiles of chunk c as 4 issues of 2 packed matmuls; adjacent
            # issues use disjoint row-group pairs so 4 run concurrently
            b = st["b"]
            qkv_full, q_rep = st["qkv_full"], st["q_rep"]
            kv_sb, k_rep = st["kv_sb"], st["k_rep"]
            ncol = c * NCH
            for j in range(4):
                s_ps = ps_s.tile([128, 2 * NCH], FP32, tag="s", name=f"s_{b}_{c}_{j}")
                for i in range(2):
                    t = 2 * j + i
                    rg = 2 * (j % 2) + i
                    ksrc = kv_sb if rg == 1 else k_rep
                    qsrc = qkv_full if rg == 0 else q_rep
                    nc.tensor.matmul(
                        s_ps[:, i * NCH : (i + 1) * NCH],
                        ksrc[32 * rg : 32 * rg + CQ, t * 128 : (t + 1) * 128],
                        qsrc[32 * rg : 32 * rg + CQ, ncol : ncol + NCH],
                        start=True,
                        stop=True,
                        tile_position=(32 * rg, 0),
                    )
                nc.scalar.activation(
                    pc[:, (2 * j) * NCH : (2 * j + 2) * NCH], s_ps[:], AFT.Exp
                )

        def av_chunk(st, ep, c, pc):
            b, vT = st["b"], st["vT"]
            o_ps = ps_av.tile([128, NCH], FP32, tag="av", name=f"av_{b}_{c}")
            for t in range(MT):
                nc.tensor.matmul(
                    o_ps[0 : CV + 1, :],
                    vT[t][:],
                    pc[:, t * NCH : (t + 1) * NCH],
                    start=(t == 0),
                    stop=(t == MT - 1),
                )
            # single drain: rows 0:64 = unnormalized AV, row 64 = denominator
            ou = eppool.tile([CV + 1, NCH], BF16, tag="ou", bufs=8,
                             name=f"ou_{b}_{c}")
            nc.vector.tensor_copy(ou[:], o_ps[0 : CV + 1, :])
            nc.sync.dma_start(ep["dstage"][c % 4 : c % 4 + 1, :], ou[CV : CV + 1, :])
            ep["ou"][c % 4] = ou

        def den_chain(ep):
            b, h = ep["bh"]
            rdn = eppool.tile([4, NCH], FP32, tag="rdn", bufs=2, name=f"rdn_{b}_{h}")
            nc.vector.reciprocal(rdn[:], ep["dstage"][:])
            ep["rd4"] = dscratch.tile([4, NCH], FP32, tag="rd", name=f"rd4_{b}_{h}")
            nc.sync.dma_start(ep["rd4"][:], rdn[:])

        def epilogue_chunk(st, ep, c):
            b, h = ep["bh"]
            x_sb = st["x_sb"]
            sl = slice(c * NCH, (c + 1) * NCH)
            if "den" not in ep:
                ep["den"] = eppool.tile([CV, 4 * NCH], FP32, tag="den", bufs=2,
                                        name=f"den_{b}_{h}")
                nc.sync.dma_start(
                    ep["den"][:].rearrange("p (c n) -> p c n", c=4, n=NCH),
                    ep["rd4"][:].partition_broadcast(CV),
                )
            onorm = eppool.tile([CV, NCH], BF16, tag="onorm", bufs=3,
                                name=f"on_{b}_{c}")
            nc.vector.scalar_tensor_tensor(
                onorm[:],
                ep["ou"][c % 4][0:CV, :],
                g_sb[0:CV, 0:1],
                ep["den"][:, (c % 4) * NCH : (c % 4 + 1) * NCH],
                AluOpType.mult,
                AluOpType.mult,
            )
            # residual folded into the wo matmul: identity-matmul x into the
            # PSUM bank (start=True), then accumulate wo@onorm on top, so the
            # DVE drain is a plain copy instead of a tensor add
            o2_ps = ps_w.tile([128, NCH], FP32, tag="wm", name=f"o2_{b}_{c}")
            nc.tensor.matmul(
                o2_ps[:], ident_bf[:], st["x_bf"][:, sl], start=True, stop=False
            )
            nc.tensor.matmul(o2_ps[:], woT[:], onorm[:], start=False, stop=True)
            out_sb = outpool.tile([C, NCH], FP32, tag="out", name=f"os_{b}_{c}")
            nc.vector.tensor_copy(out_sb[:], o2_ps[:])
            nc.gpsimd.dma_start(out_e[b, :, sl], out_sb[:])

        # ---- emission ----
        x0 = load_x(0)
        x1 = load_x(1)
        st0 = prep_init(0, x0)
        st1 = prep_init(1, x1)
        for cc in range(NCHUNKS):
            prep_chunk(st0, cc, with_vt=True)
        sts = {0: st0, 1: st1}

        av_q = []      # (st, ep, c, pc) awaiting AV
        ep_q = []      # (st, ep, c) awaiting epilogue
        eps = {}
        for i in range(2 * NCHUNKS):
            b, c = i // 8, i % 8
            st = sts[b]
            g = (b, c // 4)
            if g not in eps:
                eps[g] = {
                    "bh": g,
                    "dstage": eppool.tile([4, NCH], BF16, tag="dstage", bufs=3,
                                          name=f"dst_{g[0]}_{g[1]}"),
                    "ou": {},
                }
            if 4 <= i <= 7:
                prep_chunk(st1, 2 * (i - 4), with_vt=True)
                prep_chunk(st1, 2 * (i - 4) + 1, with_vt=True)
            pc = ppool.tile([128, HW], BF16, tag="pc", name=f"pc_{b}_{c}")
            scores_chunk(st, c, pc)
            av_q.append((st, eps[g], c, pc))
            if i >= 6:
                pops = min(2 if len(av_q) >= 2 else 1, len(av_q))
                for _ in range(pops):
                    pst, pep, pcn, ppc = av_q.pop(0)
                    av_chunk(pst, pep, pcn, ppc)
                    if pcn % 4 == 3:
                        den_chain(pep)
                        for ec in range(pcn - 3, pcn + 1):
                            ep_q.append((pst, pep, ec, i + 2))
            if ep_q and ep_q[0][3] <= i:
                e = ep_q.pop(0)
                epilogue_chunk(e[0], e[1], e[2])
        # drain remaining work
        while av_q:
            pst, pep, pcn, ppc = av_q.pop(0)
            av_chunk(pst, pep, pcn, ppc)
            if pcn % 4 == 3:
                den_chain(pep)
                for ec in range(pcn - 3, pcn + 1):
                    ep_q.append((pst, pep, ec, 0))
        while ep_q:
            e = ep_q.pop(0)
            epilogue_chunk(e[0], e[1], e[2])

    nc.finalize()
    return nc


_NC_CACHE = None


def _get_nc():
    global _NC_CACHE
    if _NC_CACHE is None:
        _NC_CACHE = build_nc()
    return _NC_CACHE


def kernel(**inputs) -> np.ndarray:
    from concourse.bass_utils import run_bass_kernel_spmd

    x = np.asarray(inputs["x"], dtype=np.float32).reshape(B_FULL, C, HW)
    wq = np.asarray(inputs["wq"], dtype=np.float32)
    wk = np.asarray(inputs["wk"], dtype=np.float32)
    wv = np.asarray(inputs["wv"], dtype=np.float32)
    wo = np.asarray(inputs["wo"], dtype=np.float32)
    gamma = np.asarray(inputs["gamma"], dtype=np.float32)

    nc = _get_nc()
    in_maps = []
    for i in range(N_CORES):
        in_maps.append(
            {
                "x": np.ascontiguousarray(x[i * B_LOC : (i + 1) * B_LOC]),
                "wq": wq,
                "wk": wk,
                "wv": wv,
                "wo": wo,
                "gamma": gamma,
            }
        )
    res = run_bass_kernel_spmd(nc, in_maps, core_ids=list(range(N_CORES)))
    outs = [res.results[i]["out"].reshape(B_LOC, C, H, W) for i in range(N_CORES)]
    return np.concatenate(outs, axis=0)


if __name__ == "__main__":
    import reference

    inputs = {k: np.asarray(v) for k, v in reference.setup_inputs().items()}
    expected = np.asarray(reference.reference(**inputs))
    actual = kernel(**inputs)
    err = np.linalg.norm(actual - expected) / np.linalg.norm(expected)
    print("Relative error:", err)
